# revision 7
# baseline (speedup 1.0000x reference)
"""Trainium2 Bass kernel for nn_ReasonerModel (12-layer cross-attn transformer).

Sharding: pure data-parallel over batch. 32 batch elems / 8 cores = 4 per core.
Each core holds the full weights (streamed from its HBM) and computes its 4
batch rows end-to-end; no collectives. Matmuls run in bf16 with fp32 PSUM
accumulation; the residual stream / layernorms stay fp32.

Layout conventions per core (B_loc = 4, SQ = 80, SKV = 1024, D = 1024, H = 16):
  x_b      [80, 1024] f32, per b    natural residual stream (LN-friendly)
  hT/pT    [128, 8, 4, 80] bf16     transposed activations (d on partitions)
  knowT    [4, 8, 128, 1024] bf16   pre-transposed know (DRAM, built in prologue)
  kT_b     [128, 8, 1024] bf16      per-b K^T   (n on partitions, s free)
  v_b      [128, 8, 1024] bf16      per-b V     (s on partitions, n free)
  aT       [128, 8, 4, 80] bf16     attention out, transposed
  gT       [128, 32, 4, 80] bf16    2*gelu(fc) transposed (0.5 folded into Wm)
All projections compute out^T = W^T-tiles @ xT so biases land on partitions.
"""

import os
import sys

sys.path.insert(0, "/opt/trn_rl_repo")

import numpy as np

import concourse.bass as bass
import concourse.tile as tile
from concourse import mybir
from concourse.bass_utils import run_bass_kernel_spmd
from concourse.masks import make_identity
from concourse.vector_clock import ScopedClock

# model dims (fixed by the problem)
B, SQ, SKV, D, H = 32, 80, 1024, 1024, 16
L = int(os.environ.get("KERNEL_LAYERS", "12"))
HD = D // H          # 64
N_CORES = 8
BL = B // N_CORES    # 4 batch rows per core
DT = D // 128        # 8 d-tiles
FT = 4 * D // 128    # 32 ffn tiles
EPS = 1e-5
GELU_C = 0.044715
GELU_S = 0.7978845608028654  # sqrt(2/pi)

F32 = mybir.dt.float32
BF16 = mybir.dt.bfloat16
AF = mybir.ActivationFunctionType
ALU = mybir.AluOpType
AX = mybir.AxisListType


class PatchedTC(tile.TileContext):
    """This container's walrus accepts at most ONE sem wait per instruction;
    Tile may attach several. Peel extras onto preceding same-engine no-ops."""

    def _commit_instruction(self, inst, lazy_reg_writes: bool = True):
        si = getattr(inst, "sync_info", None)
        if (
            si is not None
            and si.on_wait
            and len(si.on_wait) > 1
            and inst.engine != mybir.EngineType.Unassigned
        ):
            waits = list(si.on_wait)
            si.on_wait = [waits[-1]]
            for j, w in enumerate(waits[:-1]):
                nop = mybir.InstNoOp(
                    name=f"{inst.name}-sw{j}",
                    sync_info=mybir.SyncInfo(on_wait=[w], on_update=[]),
                    bass_nofuse=True,
                    engine=inst.engine,
                )
                super()._commit_instruction(nop, lazy_reg_writes=False)
        return super()._commit_instruction(inst, lazy_reg_writes)

    def _drain_and_barrier(self, tick_clock, wait_clock):
        drain_inst = self.nc.sync.drain()
        wait_clock.add_sem_waits(
            drain_inst.ins, ScopedClock({None: tick_clock.global_clock})
        )
        si = drain_inst.ins.sync_info
        if si is not None and si.on_wait and len(si.on_wait) > 1:
            waits = list(si.on_wait)
            si.on_wait = waits[:1]
            for w in waits[1:]:
                extra = self.nc.sync.drain()
                nsi = extra.ins.sync_info
                if nsi is None:
                    extra.ins.sync_info = mybir.SyncInfo(on_wait=[w], on_update=[])
                else:
                    nsi.on_wait = [w]
        self.nc.all_engine_barrier()
        assert self.sems is not None
        popped = self.nc._tile_sem_poison_stack.pop()
        assert popped is self._sem_poison
        self.nc.clear_and_free_semaphores(list(self.sems.allocated().values()))
        self.nc.all_engine_barrier()


def bcast_ap(ap_1d, p):
    """Partition-broadcast a 1-D DRAM AP to [p, n] (stride-0 partition dim)."""
    return bass.AP(
        tensor=ap_1d.tensor, offset=ap_1d.offset, ap=[[0, p]] + list(ap_1d.ap)
    )


def build_nc():
    try:  # lift the stale 192KB/partition SBUF cap to the real usable 208KB
        from concourse import tile_utils

        tile_utils.max_sbuf_usage = 208 * 1024
    except Exception:
        pass

    nc = bass.Bass("TRN2", target_bir_lowering=False, debug=False,
                   num_devices=N_CORES)

    # ---- DRAM I/O (per-core shard for acts, replicated weights) ----
    x_in = nc.dram_tensor("input_ids", [BL, SQ, D], F32, kind="ExternalInput")
    know_in = nc.dram_tensor("input_ids_know", [BL, SKV, D], F32,
                             kind="ExternalInput")
    pos_in = nc.dram_tensor("pos_embed", [SQ, D], F32, kind="ExternalInput")
    Wa = nc.dram_tensor("W_attn", [L, D, 3 * D], F32, kind="ExternalInput")
    ba = nc.dram_tensor("b_attn", [L, 3 * D], F32, kind="ExternalInput")
    Wp = nc.dram_tensor("W_proj_attn", [L, D, D], F32, kind="ExternalInput")
    bp = nc.dram_tensor("b_proj_attn", [L, D], F32, kind="ExternalInput")
    g1 = nc.dram_tensor("ln1_g", [L, D], F32, kind="ExternalInput")
    b1 = nc.dram_tensor("ln1_b", [L, D], F32, kind="ExternalInput")
    Wf = nc.dram_tensor("W_fc", [L, D, 4 * D], F32, kind="ExternalInput")
    bf = nc.dram_tensor("b_fc", [L, 4 * D], F32, kind="ExternalInput")
    Wm = nc.dram_tensor("W_proj_mlp", [L, 4 * D, D], F32, kind="ExternalInput")
    bm = nc.dram_tensor("b_proj_mlp", [L, D], F32, kind="ExternalInput")
    g2 = nc.dram_tensor("ln2_g", [L, D], F32, kind="ExternalInput")
    b2 = nc.dram_tensor("ln2_b", [L, D], F32, kind="ExternalInput")
    out_ext = nc.dram_tensor("out", [BL, SQ, D], F32, kind="ExternalOutput")

    knowT_dram = nc.dram_tensor("knowT", [BL, DT, 128, SKV], BF16)

    with PatchedTC(nc) as tc:
        import contextlib

        ctx = contextlib.ExitStack()
        with ctx:
            P = lambda **kw: ctx.enter_context(tc.tile_pool(**kw))
            singles = P(name="singles", bufs=1)
            xT_pool = P(name="xT", bufs=2)
            aT_pool = P(name="aT", bufs=1)
            oT_pool = P(name="oT", bufs=1)          # aoutT / moutT
            gT_pool = P(name="gT", bufs=1)
            kv_pool = P(name="kv", bufs=1)
            knb_pool = P(name="knb", bufs=1)
            wkv_pool = P(name="wkv", bufs=1)
            wch_pool = P(name="wch", bufs=2)        # wp/wf/wm chunks by tag
            stg_pool = P(name="stg", bufs=2)
            w_pool = P(name="wsm", bufs=2)          # softmax weights
            wT_pool = P(name="wT", bufs=4)
            tt_pool = P(name="tt", bufs=3)          # [128,128] transpose bounce
            gel_pool = P(name="gel", bufs=2)
            st_pool = P(name="st", bufs=6)          # tiny stats tiles
            bc_pool = P(name="bc", bufs=1)          # per-layer bcast vectors
            sb_pool = P(name="sb", bufs=2)          # per-layer small biases
            psA = P(name="psA", bufs=3, space="PSUM")
            psB = P(name="psB", bufs=2, space="PSUM")

            # ---- constants ----
            id_bf = singles.tile([128, 128], BF16)
            make_identity(nc, id_bf)
            id_f32 = singles.tile([128, 128], F32)
            make_identity(nc, id_f32)
            eps_t = singles.tile([128, 1], F32)
            nc.vector.memset(eps_t, EPS)

            # ---- residual-stream tiles (persistent) ----
            xs = [
                singles.tile([SQ, D], F32, tag=f"x{b}", name=f"x{b}")
                for b in range(BL)
            ]

            def ln(x_b, g_bc, b_bc):
                stt = st_pool.tile([SQ, 2, 6], F32, tag="bnst")
                mv = st_pool.tile([SQ, 2], F32, tag="bnmv")
                for c in range(2):
                    nc.vector.bn_stats(stt[:, c, :], x_b[:, c * 512:(c + 1) * 512])
                nc.vector.bn_aggr(mv, stt)
                std = st_pool.tile([SQ, 1], F32, tag="bnsd")
                nc.scalar.activation(std, mv[:, 1:2], AF.Sqrt, bias=eps_t[:SQ])
                nc.vector.reciprocal(std, std)
                nc.vector.tensor_scalar(x_b, x_b, mv[:, 0:1], std,
                                        op0=ALU.subtract, op1=ALU.mult)
                nc.vector.tensor_tensor(x_b, x_b, g_bc[:SQ, :], ALU.mult)
                nc.vector.tensor_tensor(x_b, x_b, b_bc[:SQ, :], ALU.add)

            def transpose_nat_to_T(x_b, dstT, b, cast_pool):
                """x_b [80, 1024] f32 -> dstT[:, dt, b, :] bf16 (PE transpose)."""
                for dt in range(DT):
                    pt = psA.tile([128, 512], F32, tag="psA")
                    nc.tensor.transpose(pt[:, :SQ], x_b[:, dt * 128:(dt + 1) * 128],
                                        id_f32[:SQ, :SQ])
                    eng = nc.vector if dt % 2 == 0 else nc.scalar
                    if eng is nc.vector:
                        nc.vector.tensor_copy(out=dstT[:, dt, b, :], in_=pt[:, :SQ])
                    else:
                        nc.scalar.copy(out=dstT[:, dt, b, :], in_=pt[:, :SQ])

            # ================= prologue =================
            pos_sb = singles.tile([SQ, D], F32, tag="pos")
            nc.sync.dma_start(out=pos_sb, in_=pos_in[:, :])
            hT = xT_pool.tile([128, DT, BL, SQ], BF16, tag="xT")
            for b in range(BL):
                nc.sync.dma_start(out=xs[b], in_=x_in[b])
                nc.vector.tensor_add(xs[b], xs[b], pos_sb)
                transpose_nat_to_T(xs[b], hT, b, tt_pool)

            # pre-transpose know -> knowT_dram (bf16)
            for b in range(BL):
                for stt in range(DT):
                    stg = stg_pool.tile([128, D], F32, tag="stg")
                    nc.sync.dma_start(
                        out=stg, in_=know_in[b, stt * 128:(stt + 1) * 128, :])
                    ktmp = w_pool.tile([128, D], BF16, tag="w")
                    nc.vector.tensor_copy(out=ktmp, in_=stg)
                    for dt in range(DT):
                        pt = psA.tile([128, 512], BF16, tag="psA")
                        nc.tensor.transpose(
                            pt[:, :128], ktmp[:, dt * 128:(dt + 1) * 128], id_bf)
                        kout = tt_pool.tile([128, 128], BF16, tag="tt")
                        if dt % 2 == 0:
                            nc.vector.tensor_copy(out=kout, in_=pt[:, :128])
                        else:
                            nc.scalar.copy(out=kout, in_=pt[:, :128])
                        nc.sync.dma_start(
                            out=knowT_dram[b, dt, :, stt * 128:(stt + 1) * 128],
                            in_=kout)

            # ================= layers =================
            for l in range(L):
                # ---- per-layer broadcast / bias tiles ----
                def bvec(src_ap, tag):  # [D] f32 -> [128, D] bf16 broadcast
                    stg = stg_pool.tile([128, D], F32, tag="stg")
                    nc.gpsimd.dma_start(out=stg, in_=bcast_ap(src_ap, 128))
                    t = bc_pool.tile([128, D], BF16, tag=tag)
                    nc.gpsimd.tensor_copy(out=t, in_=stg)
                    return t

                bv_bc = bvec(ba[l, 2 * D:3 * D], "bv")
                g1_bc = bvec(g1[l], "g1")
                b1_bc = bvec(b1[l], "b1")
                g2_bc = bvec(g2[l], "g2")
                b2_bc = bvec(b2[l], "b2")
                bk_sb = sb_pool.tile([128, DT], F32, tag="bk")
                nc.sync.dma_start(
                    out=bk_sb, in_=ba[l, D:2 * D].rearrange("(t p) -> p t", p=128))
                bp_sb = sb_pool.tile([128, DT], F32, tag="bp")
                nc.sync.dma_start(
                    out=bp_sb, in_=bp[l].rearrange("(t p) -> p t", p=128))
                bm_sb = sb_pool.tile([128, DT], F32, tag="bm")
                nc.sync.dma_start(
                    out=bm_sb, in_=bm[l].rearrange("(t p) -> p t", p=128))
                bf_sb = sb_pool.tile([128, FT], F32, tag="bf")
                nc.sync.dma_start(
                    out=bf_sb, in_=bf[l].rearrange("(t p) -> p t", p=128))

                # ---- stream Wk/Wv (bf16, full per layer) ----
                wk_sb = wkv_pool.tile([128, DT, D], BF16, tag="wk")
                wv_sb = wkv_pool.tile([128, DT, D], BF16, tag="wv")
                Wa_l = Wa[l].rearrange("(t p) n -> p t n", p=128)  # [128,8,3D]
                for c in range(DT):
                    stg = stg_pool.tile([128, DT, 128], F32, tag="stg")
                    nc.sync.dma_start(
                        out=stg, in_=Wa_l[:, :, D + c * 128:D + (c + 1) * 128])
                    if c % 2 == 0:
                        nc.vector.tensor_copy(
                            out=wk_sb[:, :, c * 128:(c + 1) * 128], in_=stg)
                    else:
                        nc.scalar.copy(
                            out=wk_sb[:, :, c * 128:(c + 1) * 128], in_=stg)
                for c in range(DT):
                    stg = stg_pool.tile([128, DT, 128], F32, tag="stg")
                    nc.sync.dma_start(
                        out=stg,
                        in_=Wa_l[:, :, 2 * D + c * 128:2 * D + (c + 1) * 128])
                    if c % 2 == 0:
                        nc.scalar.copy(
                            out=wv_sb[:, :, c * 128:(c + 1) * 128], in_=stg)
                    else:
                        nc.vector.tensor_copy(
                            out=wv_sb[:, :, c * 128:(c + 1) * 128], in_=stg)

                aT = aT_pool.tile([128, DT, BL, SQ], BF16, tag="aT")

                # ---- per-batch kv + attention ----
                for b in range(BL):
                    knb = knb_pool.tile([128, DT, SKV], BF16, tag="knb")
                    nc.sync.dma_start(
                        out=knb, in_=knowT_dram[b].rearrange("t p s -> p t s"))

                    # K^T: [n-part, s]
                    kTb = kv_pool.tile([128, DT, SKV], BF16, tag="kT")
                    for nt in range(DT):
                        for sc in range(2):
                            ps = psA.tile([128, 512], F32, tag="psA")
                            for kt in range(DT):
                                nc.tensor.matmul(
                                    ps,
                                    lhsT=wk_sb[:, kt, nt * 128:(nt + 1) * 128],
                                    rhs=knb[:, kt, sc * 512:(sc + 1) * 512],
                                    start=(kt == 0), stop=(kt == DT - 1))
                            nc.vector.tensor_scalar_add(
                                kTb[:, nt, sc * 512:(sc + 1) * 512], ps,
                                bk_sb[:, nt:nt + 1])

                    # V: [s-part, n]
                    vb = kv_pool.tile([128, DT, D], BF16, tag="v")
                    for stv in range(DT):
                        for nc2 in range(2):
                            ps = psA.tile([128, 512], F32, tag="psA")
                            for kt in range(DT):
                                nc.tensor.matmul(
                                    ps,
                                    lhsT=knb[:, kt, stv * 128:(stv + 1) * 128],
                                    rhs=wv_sb[:, kt, nc2 * 512:(nc2 + 1) * 512],
                                    start=(kt == 0), stop=(kt == DT - 1))
                            nc.vector.tensor_tensor(
                                vb[:, stv, nc2 * 512:(nc2 + 1) * 512], ps,
                                bv_bc[:, nc2 * 512:(nc2 + 1) * 512], ALU.add)

                    # attention, head-pair at a time
                    for hp in range(DT):
                        wTs = []
                        for hs in range(2):
                            po = hs * 64
                            scp = psB.tile([SQ, 2, 512], F32, tag="psB")
                            for sc in range(2):
                                nc.tensor.matmul(
                                    scp[:, sc, :],
                                    lhsT=hT[po:po + 64, hp, b, :],
                                    rhs=kTb[po:po + 64, hp,
                                            sc * 512:(sc + 1) * 512],
                                    start=True, stop=True)
                            sume = st_pool.tile([SQ, 1], F32, tag="sume")
                            w_sb = w_pool.tile([SQ, SKV], BF16, tag="w")
                            nc.scalar.activation(
                                out=w_sb, in_=scp.rearrange("p a s -> p (a s)"),
                                func=AF.Exp, scale=1.0 / np.sqrt(HD),
                                accum_out=sume)
                            rec = st_pool.tile([SQ, 1], F32, tag="rec")
                            nc.vector.reciprocal(rec, sume)
                            nc.vector.tensor_scalar_mul(w_sb, w_sb, rec)
                            # transpose w -> wT [s-part, st, qp]
                            wTt = wT_pool.tile([128, DT, SQ], BF16, tag="wT")
                            for g in range(2):
                                pt = psA.tile([128, 512], BF16, tag="psA")
                                for j in range(4):
                                    stw = g * 4 + j
                                    nc.tensor.transpose(
                                        pt[:, j * SQ:(j + 1) * SQ],
                                        w_sb[:, stw * 128:(stw + 1) * 128],
                                        id_bf[:SQ, :SQ])
                                src = pt[:, :4 * SQ].rearrange(
                                    "p (j q) -> p j q", j=4)
                                if g == 0:
                                    nc.vector.tensor_copy(
                                        out=wTt[:, 0:4, :], in_=src)
                                else:
                                    nc.scalar.copy(out=wTt[:, 4:8, :], in_=src)
                            wTs.append(wTt)
                        # AV for the pair: out [128, 80] (two heads on partitions)
                        pav = psA.tile([128, 512], F32, tag="psA")
                        for hs in range(2):
                            h = 2 * hp + hs
                            tp = (0, 64) if hs == 1 else None
                            for stv in range(DT):
                                nc.tensor.matmul(
                                    pav[hs * 64:(hs + 1) * 64, :SQ],
                                    lhsT=vb[:, stv, h * 64:(h + 1) * 64],
                                    rhs=wTs[hs][:, stv, :],
                                    start=(stv == 0), stop=(stv == DT - 1),
                                    tile_position=tp)
                        nc.vector.tensor_copy(out=aT[:, hp, b, :],
                                              in_=pav[:, :SQ])

                # ---- attention out-projection (out^T) ----
                aoT = oT_pool.tile([128, DT, BL, SQ], BF16, tag="oT")
                Wp_l = Wp[l].rearrange("(t p) n -> p t n", p=128)
                for nt in range(DT):
                    wpc = wch_pool.tile([128, DT, 128], BF16, tag="wp")
                    stg = stg_pool.tile([128, DT, 128], F32, tag="stg")
                    nc.sync.dma_start(
                        out=stg, in_=Wp_l[:, :, nt * 128:(nt + 1) * 128])
                    nc.vector.tensor_copy(out=wpc, in_=stg)
                    pp = psA.tile([128, 512], F32, tag="psA")
                    for kt in range(DT):
                        nc.tensor.matmul(
                            pp[:, :BL * SQ],
                            lhsT=wpc[:, kt, :],
                            rhs=aT[:, kt, :, :],
                            start=(kt == 0), stop=(kt == DT - 1))
                    nc.scalar.activation(
                        out=aoT[:, nt, :, :],
                        in_=pp[:, :BL * SQ].rearrange("p (b q) -> p b q", b=BL),
                        func=AF.Identity, bias=bp_sb[:, nt:nt + 1])

                # ---- back to natural + residual + LN1 + pT ----
                pT = xT_pool.tile([128, DT, BL, SQ], BF16, tag="xT")
                for b in range(BL):
                    for nt in range(DT):
                        pt = psA.tile([128, 512], BF16, tag="psA")
                        nc.tensor.transpose(pt[:SQ, :128], aoT[:, nt, b, :],
                                            id_bf[:128, :128])
                        nc.vector.tensor_add(
                            xs[b][:, nt * 128:(nt + 1) * 128],
                            xs[b][:, nt * 128:(nt + 1) * 128], pt[:SQ, :128])
                    ln(xs[b], g1_bc, b1_bc)
                    transpose_nat_to_T(xs[b], pT, b, tt_pool)

                # ---- ffn in (out^T) + gelu ----
                gT = gT_pool.tile([128, FT, BL, SQ], BF16, tag="gT")
                Wf_l = Wf[l].rearrange("(t p) n -> p t n", p=128)
                for nt in range(FT):
                    wfc = wch_pool.tile([128, DT, 128], BF16, tag="wf")
                    stg = stg_pool.tile([128, DT, 128], F32, tag="stg")
                    nc.sync.dma_start(
                        out=stg, in_=Wf_l[:, :, nt * 128:(nt + 1) * 128])
                    if nt % 2 == 0:
                        nc.vector.tensor_copy(out=wfc, in_=stg)
                    else:
                        nc.scalar.copy(out=wfc, in_=stg)
                    pf = psA.tile([128, 512], F32, tag="psA")
                    for kt in range(DT):
                        nc.tensor.matmul(
                            pf[:, :BL * SQ],
                            lhsT=wfc[:, kt, :],
                            rhs=pT[:, kt, :, :],
                            start=(kt == 0), stop=(kt == DT - 1))
                    # t = x + bias;  gT = (tanh(GELU_S*(t + GELU_C t^3)) + 1)*t
                    xg = gel_pool.tile([128, BL * SQ], F32, tag="gx")
                    nc.scalar.activation(out=xg, in_=pf[:, :BL * SQ],
                                         func=AF.Identity,
                                         bias=bf_sb[:, nt:nt + 1])
                    u = gel_pool.tile([128, BL * SQ], F32, tag="gu")
                    nc.vector.tensor_mul(u, xg, xg)
                    nc.vector.tensor_mul(u, u, xg)
                    nc.vector.scalar_tensor_tensor(
                        out=u, in0=u, scalar=GELU_C, in1=xg,
                        op0=ALU.mult, op1=ALU.add)
                    nc.scalar.activation(out=u, in_=u, func=AF.Tanh,
                                         scale=GELU_S)
                    nc.vector.scalar_tensor_tensor(
                        out=gT[:, nt, :, :].rearrange("p b q -> p (b q)"),
                        in0=u, scalar=1.0, in1=xg, op0=ALU.add, op1=ALU.mult)

                # ---- ffn out (out^T), 0.5 folded into Wm cast ----
                moT = oT_pool.tile([128, DT, BL, SQ], BF16, tag="oT")
                Wm_l = Wm[l].rearrange("(t p) n -> p t n", p=128)  # [128,32,D]
                for nt in range(DT):
                    wmc = wch_pool.tile([128, FT, 128], BF16, tag="wm")
                    for q in range(4):
                        stg = stg_pool.tile([128, DT, 128], F32, tag="stg")
                        nc.sync.dma_start(
                            out=stg,
                            in_=Wm_l[:, 8 * q:8 * (q + 1),
                                     nt * 128:(nt + 1) * 128])
                        nc.scalar.activation(
                            out=wmc[:, 8 * q:8 * (q + 1), :], in_=stg,
                            func=AF.Copy, scale=0.5)
                    pm = psA.tile([128, 512], F32, tag="psA")
                    for kt in range(FT):
                        nc.tensor.matmul(
                            pm[:, :BL * SQ],
                            lhsT=wmc[:, kt, :],
                            rhs=gT[:, kt, :, :],
                            start=(kt == 0), stop=(kt == FT - 1))
                    nc.scalar.activation(
                        out=moT[:, nt, :, :],
                        in_=pm[:, :BL * SQ].rearrange("p (b q) -> p b q", b=BL),
                        func=AF.Identity, bias=bm_sb[:, nt:nt + 1])

                # ---- natural + residual + LN2 + hT for next layer ----
                if l < L - 1:
                    hT = xT_pool.tile([128, DT, BL, SQ], BF16, tag="xT")
                for b in range(BL):
                    for nt in range(DT):
                        pt = psA.tile([128, 512], BF16, tag="psA")
                        nc.tensor.transpose(pt[:SQ, :128], moT[:, nt, b, :],
                                            id_bf[:128, :128])
                        nc.vector.tensor_add(
                            xs[b][:, nt * 128:(nt + 1) * 128],
                            xs[b][:, nt * 128:(nt + 1) * 128], pt[:SQ, :128])
                    ln(xs[b], g2_bc, b2_bc)
                    if l < L - 1:
                        transpose_nat_to_T(xs[b], hT, b, tt_pool)
                    else:
                        nc.sync.dma_start(out=out_ext[b], in_=xs[b])

    return nc


_CACHE = {}


def kernel(**inputs):
    if "nc" not in _CACHE:
        _CACHE["nc"] = build_nc()
    nc = _CACHE["nc"]

    x = np.ascontiguousarray(inputs["input_ids"], dtype=np.float32)
    know = np.ascontiguousarray(inputs["input_ids_know"], dtype=np.float32)
    shared = {
        "pos_embed": np.ascontiguousarray(inputs["pos_embed"], np.float32),
        "W_attn": np.ascontiguousarray(inputs["W_attn"], np.float32)[:L],
        "b_attn": np.ascontiguousarray(inputs["b_attn"], np.float32)[:L],
        "W_proj_attn": np.ascontiguousarray(inputs["W_proj_attn"], np.float32)[:L],
        "b_proj_attn": np.ascontiguousarray(inputs["b_proj_attn"], np.float32)[:L],
        "ln1_g": np.ascontiguousarray(inputs["ln1_g"], np.float32)[:L],
        "ln1_b": np.ascontiguousarray(inputs["ln1_b"], np.float32)[:L],
        "W_fc": np.ascontiguousarray(inputs["W_fc"], np.float32)[:L],
        "b_fc": np.ascontiguousarray(inputs["b_fc"], np.float32)[:L],
        "W_proj_mlp": np.ascontiguousarray(inputs["W_proj_mlp"], np.float32)[:L],
        "b_proj_mlp": np.ascontiguousarray(inputs["b_proj_mlp"], np.float32)[:L],
        "ln2_g": np.ascontiguousarray(inputs["ln2_g"], np.float32)[:L],
        "ln2_b": np.ascontiguousarray(inputs["ln2_b"], np.float32)[:L],
    }
    in_maps = []
    for i in range(N_CORES):
        m = dict(shared)
        m["input_ids"] = x[i * BL:(i + 1) * BL]
        m["input_ids_know"] = know[i * BL:(i + 1) * BL]
        in_maps.append(m)

    res = run_bass_kernel_spmd(nc, in_maps, list(range(N_CORES)))
    out = np.concatenate([res.results[i]["out"] for i in range(N_CORES)], axis=0)
    return out.astype(np.float32)
